# revision 19
# baseline (speedup 1.0000x reference)
"""Trainium2 Bass kernel for nn_AttentionBlock (sparse 7x7 windowed per-channel attention).

Semantics (validated vs reference): the torch-faithful scrambled reshape makes this,
in original coordinates, a per-(b, c, h, w-block-of-16) attention:
  logits[kh,kw] = sum_{d<16} q[c,h,16w0+d] * kpad[c,h+kh,16w0+d+kw] + qsum*(rh[kh]+rw[kw])
  out[c,h,16w0+d] = relu( sum_{kh,kw} softmax(logits)[kh,kw] * vpad[c,h+kh,16w0+d+kw] )
where kpad/vpad = conv1x1(x)+bias inside the image and exactly bias in the pad border
(conv of zero-padded x reproduces this).

Sharding: pure data parallel over 8 cores: core j -> batch j//2, image rows
[48*(j%2), 48*(j%2)+48). Each core packs its 48 rows as 2 partition-groups of 24 rows
(partition = c + 64*g) so elementwise tap work uses all 128 partitions. k/v carry a
3-row halo per group (30 rows x 102 cols; zero-pad handled host-side in x).

Per-core pipeline (v2):
  PE:   q/k/v 1x1-conv matmuls (K=65 ones-row bias trick), per-group column sets into
        psum partitions [64g, 64g+64).
  DVE:  49 qk taps: fp16 mult (2x mode) + 4-level pairwise fp16 tree reduce (2x);
        rel term; softmax denominators; 49 AV fp16 mults (2x) + fp16 pairwise tree.
        fp16 products/partials cost ~3e-3 absmax-rel end-to-end (validated vs ref).
  ACT:  exp, half the weight-broadcast expansions, odd-shift copies, relu.
  GPSIMD: the other half of the weight-broadcast expansions.
The odd-shift copies (k2o/v2o = k2/v2 shifted one column) keep the fp16 2x mode's
4-byte alignment requirement satisfied for odd kw window offsets.
"""

import sys
import os

sys.path.insert(0, "/opt/trn_rl_repo")

import numpy as np

B, C, H, W = 4, 64, 96, 96
KS, NH = 7, 4
HALO = (KS - 1) // 2          # 3
NCORES = 8
RPC = H // 2                  # 48 rows per core
G = 2                         # partition groups per core
RPG = RPC // G                # 24 rows per group
KR = RPG + KS - 1             # 30 k/v rows per group
XR = RPC + KS - 1             # 54 x rows per core
WE = W + KS - 1               # 102 extended cols
NB = W // 16                  # 6 w-blocks
NS = RPG * NB                 # 144 sites per partition
NT = KS * KS                  # 49 taps

_cache = {}


def _build():
    import concourse.bacc as bacc
    import concourse.bass as bass
    import concourse.tile as tile
    from concourse import mybir

    f32 = mybir.dt.float32
    f16 = mybir.dt.float16
    Act = mybir.ActivationFunctionType

    nc = bacc.Bacc(
        "TRN2",
        target_bir_lowering=False,
        debug=False,
        enable_asserts=False,
        num_devices=NCORES,
    )

    xc_d = nc.dram_tensor("xc", [C + 1, XR, WE], f32, kind="ExternalInput").ap()
    wq_d = nc.dram_tensor("wq", [C + 1, C], f32, kind="ExternalInput").ap()
    wk_d = nc.dram_tensor("wk", [C + 1, C], f32, kind="ExternalInput").ap()
    wv_d = nc.dram_tensor("wv", [C + 1, C], f32, kind="ExternalInput").ap()
    rel_d = nc.dram_tensor("relv", [NT], f32, kind="ExternalInput").ap()
    out_d = nc.dram_tensor("outp", [2 * C, RPG, W], f32, kind="ExternalOutput").ap()

    from contextlib import ExitStack

    with tile.TileContext(nc) as tc:
        with ExitStack() as stk:
            wp = stk.enter_context(tc.tile_pool(name="wpool", bufs=1))
            mp = stk.enter_context(tc.tile_pool(name="main", bufs=1))
            tp = stk.enter_context(tc.tile_pool(name="tmp", bufs=2))
            pp = stk.enter_context(
                tc.tile_pool(name="psum", bufs=4, space=bass.MemorySpace.PSUM)
            )
            xp_cm = tc.tile_pool(name="xpool", bufs=1)
            xp = xp_cm.__enter__()

            # ---- loads ----
            xc = xp.tile([C + 1, XR, WE], f32)
            nc.sync.dma_start(out=xc[:, :KR, :], in_=xc_d[:, :KR, :])
            nc.sync.dma_start(out=xc[:, KR:, :], in_=xc_d[:, KR:, :])
            wq = wp.tile([C + 1, C], f32)
            wk = wp.tile([C + 1, C], f32)
            wv = wp.tile([C + 1, C], f32)
            nc.sync.dma_start(out=wq, in_=wq_d)
            nc.sync.dma_start(out=wk, in_=wk_d)
            nc.sync.dma_start(out=wv, in_=wv_d)
            relsb = wp.tile([128, NT], f32)
            nc.sync.dma_start(
                out=relsb, in_=rel_d.unsqueeze(0).broadcast_to([128, NT])
            )

            # ---- persistent tensors ----
            k2 = mp.tile([128, KR, WE], f16)    # partition = c + 64g
            v2 = mp.tile([128, KR, WE], f16)
            k2o = mp.tile([128, KR, WE], f16)   # shifted 1 col (fp16 alignment)
            v2o = mp.tile([128, KR, WE], f16)
            q2 = mp.tile([128, RPG, W], f16)
            qs = mp.tile([128, NS], f32)
            A = mp.tile([128, NT, NS], f32)     # logits+rel -> exp(wts)
            den = mp.tile([128, NS], f32)
            rden = mp.tile([128, NS], f32)
            wn16 = mp.tile([128, NT, NS], f16)  # normalized weights

            # ---- projections ----
            KVCH = 6
            kv_n = KR * WE // KVCH  # 510
            for dst, wgt in ((k2, wk), (v2, wv)):
                for ci in range(KVCH):
                    ps = pp.tile([128, 512], f32, tag="ps_kv")
                    for g in range(G):
                        rhs = (
                            xc[:, RPG * g : RPG * g + KR, :]
                            .rearrange("p a b -> p (a b)")[:, ci * kv_n : (ci + 1) * kv_n]
                        )
                        nc.tensor.matmul(
                            ps[64 * g : 64 * g + 64, :kv_n],
                            wgt,
                            rhs,
                            start=True,
                            stop=True,
                        )
                    dst_sl = dst.rearrange("p a b -> p (a b)")[
                        :, ci * kv_n : (ci + 1) * kv_n
                    ]
                    if ci % 2 == 0:
                        nc.vector.tensor_copy(dst_sl, ps[:, :kv_n])
                    else:
                        nc.scalar.copy(dst_sl, ps[:, :kv_n])
            QCH = 6
            qrows = RPG // QCH  # 4
            qn = qrows * W  # 384
            for ci in range(QCH):
                ps = pp.tile([128, 512], f32, tag="ps_q")
                for g in range(G):
                    r0 = HALO + RPG * g + ci * qrows
                    rhs = xc[:, r0 : r0 + qrows, HALO : HALO + W]
                    nc.tensor.matmul(
                        ps[64 * g : 64 * g + 64, :qn], wq, rhs, start=True, stop=True
                    )
                q2_sl = q2[:, ci * qrows : (ci + 1) * qrows, :]
                ps_sl = ps[:, :qn].rearrange("p (a b) -> p a b", b=W)
                if ci % 2 == 0:
                    nc.vector.tensor_copy(q2_sl, ps_sl)
                else:
                    nc.scalar.copy(q2_sl, ps_sl)

            # odd-shifted copies for fp16 alignment at odd kw
            nc.scalar.copy(
                k2o.rearrange("p a b -> p (a b)")[:, : KR * WE - 1],
                k2.rearrange("p a b -> p (a b)")[:, 1:],
            )
            nc.gpsimd.tensor_copy(
                v2o.rearrange("p a b -> p (a b)")[:, : KR * WE - 1],
                v2.rearrange("p a b -> p (a b)")[:, 1:],
            )

            # projections emitted; release x pool address space
            xp_cm.__exit__(None, None, None)
            abp = stk.enter_context(tc.tile_pool(name="abpool", bufs=1))

            # ---- qsum and rel outer product ----
            nc.vector.reduce_sum(
                out=qs,
                in_=q2.rearrange("p h (a b) -> p (h a) b", b=16),
                axis=mybir.AxisListType.X,
            )
            # ---- qk taps: fp16 mult (2x) + 4-level fp16 pairwise tree (2x),
            # then fused logits assembly A[:,t,:] = qs*rel_t + treesum (STT) ----
            for kh in range(KS):
                for kw in range(KS):
                    t = kh * KS + kw
                    ksrc, kwoff = (k2, kw) if kw % 2 == 0 else (k2o, kw - 1)
                    pr = tp.tile([128, RPG, W], f16, tag="pr")
                    nc.vector.tensor_mul(
                        pr, q2, ksrc[:, kh : kh + RPG, kwoff : kwoff + W]
                    )
                    prv = pr.rearrange("p h (a b) -> p (h a) b", b=16)  # [128,144,16]
                    t1 = tp.tile([128, NS, 8], f16, tag="t1")
                    nc.vector.tensor_add(t1, prv[:, :, 0:8], prv[:, :, 8:16])
                    t2 = tp.tile([128, NS, 4], f16, tag="t2")
                    nc.vector.tensor_add(t2, t1[:, :, 0:4], t1[:, :, 4:8])
                    t3 = tp.tile([128, NS, 2], f16, tag="t3")
                    nc.vector.tensor_add(t3, t2[:, :, 0:2], t2[:, :, 2:4])
                    l4 = tp.tile([128, NS], f32, tag="l4")
                    nc.vector.tensor_add(l4, t3[:, :, 0], t3[:, :, 1])
                    nc.vector.scalar_tensor_tensor(
                        out=A[:, t, :],
                        in0=qs,
                        scalar=relsb[:, t : t + 1],
                        in1=l4,
                        op0=mybir.AluOpType.mult,
                        op1=mybir.AluOpType.add,
                    )
            # exp (no max-subtraction; |logits| < ~55 so fp32-safe)
            nc.scalar.activation(A, A, Act.Exp)
            # denominators over taps, reciprocal, normalized fp16 weights
            nc.vector.reduce_sum(
                out=den, in_=A.transpose([0, 2, 1]), axis=mybir.AxisListType.X
            )
            nc.vector.reciprocal(rden, den)
            nc.vector.tensor_mul(
                wn16, A, rden.unsqueeze(1).broadcast_to([128, NT, NS])
            )

            # ---- AV: expand wts (ACT/GPSIMD), fp16 mult (2x), 3-level fp16 tree
            # per 8-tap group, groups chained into an fp32 accumulator ----
            acc = mp.tile([128, RPG, W], f32)
            carry = {}  # tree level (0..2) -> partial-sum tile
            state = {"groups": 0}

            def tree_push(p, level=0):
                while level in carry and level < 3:
                    prev = carry.pop(level)
                    s = abp.tile(
                        [128, RPG, W], f16,
                        tag=f"ts{level}", name=f"ts{level}", bufs=2,
                    )
                    nc.vector.tensor_add(s, prev, p)
                    p = s
                    level += 1
                if level == 3:
                    if state["groups"] == 0:
                        nc.vector.tensor_copy(acc, p)
                    else:
                        nc.vector.tensor_add(acc, acc, p)
                    state["groups"] += 1
                else:
                    carry[level] = p

            for kh in range(KS):
                for kw in range(KS):
                    t = kh * KS + kw
                    vsrc, kwoff = (v2, kw) if kw % 2 == 0 else (v2o, kw - 1)
                    wexp = abp.tile(
                        [128, RPG, NB, 16], f16, tag="wexp", name="wexp", bufs=3
                    )
                    wsl = (
                        wn16[:, t, :]
                        .rearrange("p (h a) -> p h a", a=NB)
                        .unsqueeze(3)
                        .broadcast_to([128, RPG, NB, 16])
                    )
                    if t % 2 == 0:
                        nc.scalar.copy(wexp, wsl)
                    else:
                        nc.gpsimd.tensor_copy(wexp, wsl)
                    p = abp.tile([128, RPG, W], f16, tag="avp", name="avp", bufs=3)
                    nc.vector.tensor_mul(
                        p,
                        wexp.rearrange("p h a b -> p h (a b)"),
                        vsrc[:, kh : kh + RPG, kwoff : kwoff + W],
                    )
                    tree_push(p)
            # leftover carries (49 = 6*8 + 1 -> a level-0 carry remains)
            for lv in sorted(carry):
                nc.vector.tensor_add(acc, acc, carry.pop(lv))
            nc.scalar.activation(acc, acc, Act.Relu)
            nc.sync.dma_start(out=out_d, in_=acc)

    nc.compile()
    return nc


def _get_nc():
    if "nc" not in _cache:
        _cache["nc"] = _build()
    return _cache["nc"]


def _prep_inputs(inputs):
    """Host-side shard prep. Returns list of 8 in_maps."""
    x = np.ascontiguousarray(np.asarray(inputs["input_x"], dtype=np.float32))
    qw = np.asarray(inputs["q_w"], np.float32)
    qb = np.asarray(inputs["q_b"], np.float32)
    kw_ = np.asarray(inputs["k_w"], np.float32)
    kb = np.asarray(inputs["k_b"], np.float32)
    vw = np.asarray(inputs["v_w"], np.float32)
    vb = np.asarray(inputs["v_b"], np.float32)
    rh = np.asarray(inputs["rel_h"], np.float32).sum(0)[:, 0]  # (7,)
    rw = np.asarray(inputs["rel_w"], np.float32).sum(0)[0, :]  # (7,)

    wq = np.concatenate([qw.T, qb[None, :]], axis=0).astype(np.float32)  # (65, 64)
    wk = np.concatenate([kw_.T, kb[None, :]], axis=0).astype(np.float32)
    wv = np.concatenate([vw.T, vb[None, :]], axis=0).astype(np.float32)
    relv = (rh[:, None] + rw[None, :]).reshape(-1).astype(np.float32)  # (49,)

    # padded x with ones channel: (B, 65, 102, 102)
    xpad = np.zeros((B, C + 1, H + 2 * HALO, W + 2 * HALO), np.float32)
    xpad[:, :C, HALO : HALO + H, HALO : HALO + W] = x
    xpad[:, C, :, :] = 1.0

    in_maps = []
    for j in range(NCORES):
        b = j // 2
        r0 = RPC * (j % 2)
        xc = np.ascontiguousarray(xpad[b, :, r0 : r0 + XR, :])  # (65, 54, 102)
        in_maps.append({"xc": xc, "wq": wq, "wk": wk, "wv": wv, "relv": relv})
    return in_maps


def _assemble(results):
    """results: list of 8 dicts with 'outp' (128, 24, 96) -> (B, C, H, W)."""
    y = np.empty((B, C, H, W), np.float32)
    for j in range(NCORES):
        o = results[j]["outp"]
        b = j // 2
        r0 = RPC * (j % 2)
        for g in range(G):
            y[b, :, r0 + RPG * g : r0 + RPG * (g + 1), :] = o[64 * g : 64 * g + 64]
    return y


def _install_ntff_hook():
    """Register the axon NTFF profiling hook (the image lacks antenv.axon_hooks)."""
    import types
    import antenv

    if "antenv.axon_hooks" in sys.modules:
        return
    mod = types.ModuleType("antenv.axon_hooks")
    _state = {"hook": None}
    mod.set_axon_ntff_profile_hook = lambda h: _state.__setitem__("hook", h)
    mod.get_axon_ntff_profile_hook = lambda: _state["hook"]
    sys.modules["antenv.axon_hooks"] = mod
    antenv.axon_hooks = mod
    from trn_agent_boot.trn_boot import _ntff_profile_via_ctypes

    mod.set_axon_ntff_profile_hook(_ntff_profile_via_ctypes("/opt/axon/libaxon_pjrt.so"))
    # avoid S3 artifact uploads in-container
    from concourse import bass_utils

    bass_utils.upload_artifacts = lambda tmpdir: tmpdir


def kernel(**inputs) -> np.ndarray:
    from concourse import bass_utils

    nc = _get_nc()
    in_maps = _prep_inputs(inputs)
    trace = bool(int(os.environ.get("KERNEL_TRACE", "0")))
    kw = {}
    if trace:
        _install_ntff_hook()
        kw["tmpdir"] = os.environ.get("KERNEL_TRACE_DIR") or None
    res = bass_utils.run_bass_kernel_spmd(
        nc, in_maps, core_ids=list(range(NCORES)), trace=trace, **kw
    )
    _cache["last_result"] = res
    return _assemble(res.results)


def kernel_sim(inputs, cores=(0,)):
    """CoreSim-based check (no hardware). Returns partial output dict {core: outp}."""
    from concourse.bass_interp import CoreSim

    nc = _get_nc()
    in_maps = _prep_inputs(inputs)
    outs = {}
    for j in cores:
        sim = CoreSim(nc, trace=False, require_finite=True, require_nnan=True)
        for name, arr in in_maps[j].items():
            sim.tensor(name)[:] = arr
        sim.simulate(check_with_hw=False)
        outs[j] = np.array(sim.tensor("outp"))
    return outs


# revision 21
# speedup vs baseline: 1.4636x; 1.4636x over previous
"""Trainium2 Bass kernel for nn_AttentionBlock (sparse 7x7 windowed per-channel attention).

Semantics (validated vs reference): the torch-faithful scrambled reshape makes this,
in original coordinates, a per-(b, c, h, w-block-of-16) attention:
  logits[kh,kw] = sum_{d<16} q[c,h,16w0+d] * kpad[c,h+kh,16w0+d+kw] + qsum*(rh[kh]+rw[kw])
  out[c,h,16w0+d] = relu( sum_{kh,kw} softmax(logits)[kh,kw] * vpad[c,h+kh,16w0+d+kw] )
where kpad/vpad = conv1x1(x)+bias inside the image and exactly bias in the pad border
(conv of zero-padded x reproduces this).

Sharding: pure data parallel over 8 cores: core j -> batch j//2, image rows
[48*(j%2), 48*(j%2)+48). Each core packs its 48 rows as 2 partition-groups of 24 rows
(partition = c + 64*g) so elementwise tap work uses all 128 partitions. k/v carry a
3-row halo per group (30 rows x 102 cols; zero-pad handled host-side in x).

Per-core pipeline (v2):
  PE:   q/k/v 1x1-conv matmuls (K=65 ones-row bias trick), per-group column sets into
        psum partitions [64g, 64g+64).
  DVE:  49 qk taps: fp16 mult (2x mode) + 4-level pairwise fp16 tree reduce (2x);
        rel term; softmax denominators; 49 AV fp16 mults (2x) + fp16 pairwise tree.
        fp16 products/partials cost ~3e-3 absmax-rel end-to-end (validated vs ref).
  ACT:  exp, half the weight-broadcast expansions, odd-shift copies, relu.
  GPSIMD: the other half of the weight-broadcast expansions.
The odd-shift copies (k2o/v2o = k2/v2 shifted one column) keep the fp16 2x mode's
4-byte alignment requirement satisfied for odd kw window offsets.
"""

import sys
import os

sys.path.insert(0, "/opt/trn_rl_repo")

import numpy as np

B, C, H, W = 4, 64, 96, 96
KS, NH = 7, 4
HALO = (KS - 1) // 2          # 3
NCORES = 8
RPC = H // 2                  # 48 rows per core
G = 2                         # partition groups per core
RPG = RPC // G                # 24 rows per group
KR = RPG + KS - 1             # 30 k/v rows per group
XR = RPC + KS - 1             # 54 x rows per core
WE = W + KS - 1               # 102 extended cols
NB = W // 16                  # 6 w-blocks
NS = RPG * NB                 # 144 sites per partition
NT = KS * KS                  # 49 taps

_cache = {}


def _build():
    import concourse.bacc as bacc
    import concourse.bass as bass
    import concourse.tile as tile
    from concourse import mybir

    f32 = mybir.dt.float32
    f16 = mybir.dt.float16
    Act = mybir.ActivationFunctionType

    nc = bacc.Bacc(
        "TRN2",
        target_bir_lowering=False,
        debug=False,
        enable_asserts=False,
        num_devices=NCORES,
    )

    xc_d = nc.dram_tensor("xc", [C + 1, XR, WE], f32, kind="ExternalInput").ap()
    wq_d = nc.dram_tensor("wq", [C + 1, C], f32, kind="ExternalInput").ap()
    wk_d = nc.dram_tensor("wk", [C + 1, C], f32, kind="ExternalInput").ap()
    wv_d = nc.dram_tensor("wv", [C + 1, C], f32, kind="ExternalInput").ap()
    rel_d = nc.dram_tensor("relv", [NT], f32, kind="ExternalInput").ap()
    out_d = nc.dram_tensor("outp", [2 * C, RPG, W], f32, kind="ExternalOutput").ap()

    from contextlib import ExitStack

    with tile.TileContext(nc) as tc:
        with ExitStack() as stk:
            wp = stk.enter_context(tc.tile_pool(name="wpool", bufs=1))
            mp = stk.enter_context(tc.tile_pool(name="main", bufs=1))
            tp = stk.enter_context(tc.tile_pool(name="tmp", bufs=2))
            pp = stk.enter_context(
                tc.tile_pool(name="psum", bufs=4, space=bass.MemorySpace.PSUM)
            )
            xp_cm = tc.tile_pool(name="xpool", bufs=1)
            xp = xp_cm.__enter__()

            # ---- loads ----
            xc = xp.tile([C + 1, XR, WE], f32)
            nc.sync.dma_start(out=xc[:, :KR, :], in_=xc_d[:, :KR, :])
            nc.sync.dma_start(out=xc[:, KR:, :], in_=xc_d[:, KR:, :])
            wq = wp.tile([C + 1, C], f32)
            wk = wp.tile([C + 1, C], f32)
            wv = wp.tile([C + 1, C], f32)
            nc.sync.dma_start(out=wq, in_=wq_d)
            nc.sync.dma_start(out=wk, in_=wk_d)
            nc.sync.dma_start(out=wv, in_=wv_d)
            relsb = wp.tile([128, NT], f32)
            nc.sync.dma_start(
                out=relsb, in_=rel_d.unsqueeze(0).broadcast_to([128, NT])
            )

            # ---- persistent tensors ----
            k2 = mp.tile([128, KR, WE], f16)    # partition = c + 64g
            v2 = mp.tile([128, KR, WE], f16)
            k2o = mp.tile([128, KR, WE], f16)   # shifted 1 col (fp16 alignment)
            v2o = mp.tile([128, KR, WE], f16)
            q2 = mp.tile([128, RPG, W], f16)
            qs = mp.tile([128, NS], f32)
            A = mp.tile([128, NT, NS], f32)     # logits+rel -> exp(wts)
            den = mp.tile([128, NS], f32)
            rden = mp.tile([128, NS], f32)
            wn16 = mp.tile([128, NT, NS], f16)  # normalized weights

            # ---- projections ----
            KVCH = 6
            kv_n = KR * WE // KVCH  # 510
            for dst, wgt in ((k2, wk), (v2, wv)):
                for ci in range(KVCH):
                    ps = pp.tile([128, 512], f32, tag="ps_kv")
                    for g in range(G):
                        rhs = (
                            xc[:, RPG * g : RPG * g + KR, :]
                            .rearrange("p a b -> p (a b)")[:, ci * kv_n : (ci + 1) * kv_n]
                        )
                        nc.tensor.matmul(
                            ps[64 * g : 64 * g + 64, :kv_n],
                            wgt,
                            rhs,
                            start=True,
                            stop=True,
                        )
                    dst_sl = dst.rearrange("p a b -> p (a b)")[
                        :, ci * kv_n : (ci + 1) * kv_n
                    ]
                    if ci % 2 == 0:
                        nc.vector.tensor_copy(dst_sl, ps[:, :kv_n])
                    else:
                        nc.scalar.copy(dst_sl, ps[:, :kv_n])
            QCH = 6
            qrows = RPG // QCH  # 4
            qn = qrows * W  # 384
            for ci in range(QCH):
                ps = pp.tile([128, 512], f32, tag="ps_q")
                for g in range(G):
                    r0 = HALO + RPG * g + ci * qrows
                    rhs = xc[:, r0 : r0 + qrows, HALO : HALO + W]
                    nc.tensor.matmul(
                        ps[64 * g : 64 * g + 64, :qn], wq, rhs, start=True, stop=True
                    )
                q2_sl = q2[:, ci * qrows : (ci + 1) * qrows, :]
                ps_sl = ps[:, :qn].rearrange("p (a b) -> p a b", b=W)
                if ci % 2 == 0:
                    nc.vector.tensor_copy(q2_sl, ps_sl)
                else:
                    nc.scalar.copy(q2_sl, ps_sl)

            # odd-shifted copies for fp16 alignment at odd kw
            nc.scalar.copy(
                k2o.rearrange("p a b -> p (a b)")[:, : KR * WE - 1],
                k2.rearrange("p a b -> p (a b)")[:, 1:],
            )
            nc.scalar.copy(
                v2o.rearrange("p a b -> p (a b)")[:, : KR * WE - 1],
                v2.rearrange("p a b -> p (a b)")[:, 1:],
            )

            # projections emitted; release x pool address space
            xp_cm.__exit__(None, None, None)
            abp = stk.enter_context(tc.tile_pool(name="abpool", bufs=1))

            # ---- qsum and rel outer product ----
            nc.vector.reduce_sum(
                out=qs,
                in_=q2.rearrange("p h (a b) -> p (h a) b", b=16),
                axis=mybir.AxisListType.X,
            )
            # ---- qk taps: fp16 mult (2x) + 4-level fp16 pairwise tree (2x),
            # then fused logits assembly A[:,t,:] = qs*rel_t + treesum (STT) ----
            for kh in range(KS):
                for kw in range(KS):
                    t = kh * KS + kw
                    ksrc, kwoff = (k2, kw) if kw % 2 == 0 else (k2o, kw - 1)
                    pr = tp.tile([128, RPG, W], f16, tag="pr")
                    nc.vector.tensor_mul(
                        pr, q2, ksrc[:, kh : kh + RPG, kwoff : kwoff + W]
                    )
                    prv = pr.rearrange("p h (a b) -> p (h a) b", b=16)  # [128,144,16]
                    t1 = tp.tile([128, NS, 8], f16, tag="t1")
                    nc.vector.tensor_add(t1, prv[:, :, 0:8], prv[:, :, 8:16])
                    t2 = tp.tile([128, NS, 4], f16, tag="t2")
                    nc.vector.tensor_add(t2, t1[:, :, 0:4], t1[:, :, 4:8])
                    t3 = tp.tile([128, NS, 2], f16, tag="t3")
                    nc.vector.tensor_add(t3, t2[:, :, 0:2], t2[:, :, 2:4])
                    l4 = tp.tile([128, NS], f32, tag="l4")
                    nc.vector.tensor_add(l4, t3[:, :, 0], t3[:, :, 1])
                    nc.vector.scalar_tensor_tensor(
                        out=A[:, t, :],
                        in0=qs,
                        scalar=relsb[:, t : t + 1],
                        in1=l4,
                        op0=mybir.AluOpType.mult,
                        op1=mybir.AluOpType.add,
                    )
            # exp (no max-subtraction; |logits| < ~55 so fp32-safe)
            nc.scalar.activation(A, A, Act.Exp)
            # denominators over taps, reciprocal, normalized fp16 weights
            nc.vector.reduce_sum(
                out=den, in_=A.transpose([0, 2, 1]), axis=mybir.AxisListType.X
            )
            nc.vector.reciprocal(rden, den)
            nc.vector.tensor_mul(
                wn16, A, rden.unsqueeze(1).broadcast_to([128, NT, NS])
            )

            # ---- AV: expand wts (ACT/GPSIMD), fp16 mult (2x), 3-level fp16 tree
            # per 8-tap group, groups chained into an fp32 accumulator ----
            acc = mp.tile([128, RPG, W], f32)
            carry = {}  # tree level (0..2) -> partial-sum tile
            state = {"groups": 0}

            def tree_push(p, level=0):
                while level in carry and level < 3:
                    prev = carry.pop(level)
                    s = abp.tile(
                        [128, RPG, W], f16,
                        tag=f"ts{level}", name=f"ts{level}", bufs=2,
                    )
                    nc.vector.tensor_add(s, prev, p)
                    p = s
                    level += 1
                if level == 3:
                    if state["groups"] == 0:
                        nc.vector.tensor_copy(acc, p)
                    else:
                        nc.vector.tensor_add(acc, acc, p)
                    state["groups"] += 1
                else:
                    carry[level] = p

            for kh in range(KS):
                for kw in range(KS):
                    t = kh * KS + kw
                    vsrc, kwoff = (v2, kw) if kw % 2 == 0 else (v2o, kw - 1)
                    wexp = abp.tile(
                        [128, RPG, NB, 16], f16, tag="wexp", name="wexp", bufs=3
                    )
                    wsl = (
                        wn16[:, t, :]
                        .rearrange("p (h a) -> p h a", a=NB)
                        .unsqueeze(3)
                        .broadcast_to([128, RPG, NB, 16])
                    )
                    nc.scalar.copy(wexp, wsl)
                    p = abp.tile([128, RPG, W], f16, tag="avp", name="avp", bufs=3)
                    nc.vector.tensor_mul(
                        p,
                        wexp.rearrange("p h a b -> p h (a b)"),
                        vsrc[:, kh : kh + RPG, kwoff : kwoff + W],
                    )
                    tree_push(p)
            # leftover carries (49 = 6*8 + 1 -> a level-0 carry remains)
            for lv in sorted(carry):
                nc.vector.tensor_add(acc, acc, carry.pop(lv))
            nc.scalar.activation(acc, acc, Act.Relu)
            nc.sync.dma_start(out=out_d, in_=acc)

    nc.compile()
    return nc


def _get_nc():
    if "nc" not in _cache:
        _cache["nc"] = _build()
    return _cache["nc"]


def _prep_inputs(inputs):
    """Host-side shard prep. Returns list of 8 in_maps."""
    x = np.ascontiguousarray(np.asarray(inputs["input_x"], dtype=np.float32))
    qw = np.asarray(inputs["q_w"], np.float32)
    qb = np.asarray(inputs["q_b"], np.float32)
    kw_ = np.asarray(inputs["k_w"], np.float32)
    kb = np.asarray(inputs["k_b"], np.float32)
    vw = np.asarray(inputs["v_w"], np.float32)
    vb = np.asarray(inputs["v_b"], np.float32)
    rh = np.asarray(inputs["rel_h"], np.float32).sum(0)[:, 0]  # (7,)
    rw = np.asarray(inputs["rel_w"], np.float32).sum(0)[0, :]  # (7,)

    wq = np.concatenate([qw.T, qb[None, :]], axis=0).astype(np.float32)  # (65, 64)
    wk = np.concatenate([kw_.T, kb[None, :]], axis=0).astype(np.float32)
    wv = np.concatenate([vw.T, vb[None, :]], axis=0).astype(np.float32)
    relv = (rh[:, None] + rw[None, :]).reshape(-1).astype(np.float32)  # (49,)

    # padded x with ones channel: (B, 65, 102, 102)
    xpad = np.zeros((B, C + 1, H + 2 * HALO, W + 2 * HALO), np.float32)
    xpad[:, :C, HALO : HALO + H, HALO : HALO + W] = x
    xpad[:, C, :, :] = 1.0

    in_maps = []
    for j in range(NCORES):
        b = j // 2
        r0 = RPC * (j % 2)
        xc = np.ascontiguousarray(xpad[b, :, r0 : r0 + XR, :])  # (65, 54, 102)
        in_maps.append({"xc": xc, "wq": wq, "wk": wk, "wv": wv, "relv": relv})
    return in_maps


def _assemble(results):
    """results: list of 8 dicts with 'outp' (128, 24, 96) -> (B, C, H, W)."""
    y = np.empty((B, C, H, W), np.float32)
    for j in range(NCORES):
        o = results[j]["outp"]
        b = j // 2
        r0 = RPC * (j % 2)
        for g in range(G):
            y[b, :, r0 + RPG * g : r0 + RPG * (g + 1), :] = o[64 * g : 64 * g + 64]
    return y


def _install_ntff_hook():
    """Register the axon NTFF profiling hook (the image lacks antenv.axon_hooks)."""
    import types
    import antenv

    if "antenv.axon_hooks" in sys.modules:
        return
    mod = types.ModuleType("antenv.axon_hooks")
    _state = {"hook": None}
    mod.set_axon_ntff_profile_hook = lambda h: _state.__setitem__("hook", h)
    mod.get_axon_ntff_profile_hook = lambda: _state["hook"]
    sys.modules["antenv.axon_hooks"] = mod
    antenv.axon_hooks = mod
    from trn_agent_boot.trn_boot import _ntff_profile_via_ctypes

    mod.set_axon_ntff_profile_hook(_ntff_profile_via_ctypes("/opt/axon/libaxon_pjrt.so"))
    # avoid S3 artifact uploads in-container
    from concourse import bass_utils

    bass_utils.upload_artifacts = lambda tmpdir: tmpdir


def kernel(**inputs) -> np.ndarray:
    from concourse import bass_utils

    nc = _get_nc()
    in_maps = _prep_inputs(inputs)
    trace = bool(int(os.environ.get("KERNEL_TRACE", "0")))
    kw = {}
    if trace:
        _install_ntff_hook()
        kw["tmpdir"] = os.environ.get("KERNEL_TRACE_DIR") or None
    res = bass_utils.run_bass_kernel_spmd(
        nc, in_maps, core_ids=list(range(NCORES)), trace=trace, **kw
    )
    _cache["last_result"] = res
    return _assemble(res.results)


def kernel_sim(inputs, cores=(0,)):
    """CoreSim-based check (no hardware). Returns partial output dict {core: outp}."""
    from concourse.bass_interp import CoreSim

    nc = _get_nc()
    in_maps = _prep_inputs(inputs)
    outs = {}
    for j in cores:
        sim = CoreSim(nc, trace=False, require_finite=True, require_nnan=True)
        for name, arr in in_maps[j].items():
            sim.tensor(name)[:] = arr
        sim.simulate(check_with_hw=False)
        outs[j] = np.array(sim.tensor("outp"))
    return outs


# revision 25
# speedup vs baseline: 1.6497x; 1.1271x over previous
"""Trainium2 Bass kernel for nn_AttentionBlock (sparse 7x7 windowed per-channel attention).

Semantics (validated vs reference): the torch-faithful scrambled reshape makes this,
in original coordinates, a per-(b, c, h, w-block-of-16) attention:
  logits[kh,kw] = sum_{d<16} q[c,h,16w0+d] * kpad[c,h+kh,16w0+d+kw] + qsum*(rh[kh]+rw[kw])
  out[c,h,16w0+d] = relu( sum_{kh,kw} softmax(logits)[kh,kw] * vpad[c,h+kh,16w0+d+kw] )
where kpad/vpad = conv1x1(x)+bias inside the image and exactly bias in the pad border
(conv of zero-padded x reproduces this).

Sharding: pure data parallel over 8 cores: core j -> batch j//2, image rows
[48*(j%2), 48*(j%2)+48). Each core packs its 48 rows as 2 partition-groups of 24 rows
(partition = c + 64*g) so elementwise tap work uses all 128 partitions. k/v carry a
3-row halo per group (30 rows x 102 cols; zero-pad handled host-side in x).

Per-core pipeline (v2):
  PE:   q/k/v 1x1-conv matmuls (K=65 ones-row bias trick), per-group column sets into
        psum partitions [64g, 64g+64).
  DVE:  49 qk taps: fp16 mult (2x mode) + 4-level pairwise fp16 tree reduce (2x);
        rel term; softmax denominators; 49 AV fp16 mults (2x) + fp16 pairwise tree.
        fp16 products/partials cost ~3e-3 absmax-rel end-to-end (validated vs ref).
  ACT:  exp, half the weight-broadcast expansions, odd-shift copies, relu.
  GPSIMD: the other half of the weight-broadcast expansions.
The odd-shift copies (k2o/v2o = k2/v2 shifted one column) keep the fp16 2x mode's
4-byte alignment requirement satisfied for odd kw window offsets.
"""

import sys
import os

sys.path.insert(0, "/opt/trn_rl_repo")

import numpy as np

B, C, H, W = 4, 64, 96, 96
KS, NH = 7, 4
HALO = (KS - 1) // 2          # 3
NCORES = 8
RPC = H // 2                  # 48 rows per core
G = 2                         # partition groups per core
RPG = RPC // G                # 24 rows per group
KR = RPG + KS - 1             # 30 k/v rows per group
XR = RPC + KS - 1             # 54 x rows per core
WE = W + KS - 1               # 102 extended cols
NB = W // 16                  # 6 w-blocks
NS = RPG * NB                 # 144 sites per partition
NT = KS * KS                  # 49 taps

_cache = {}


def _build():
    import concourse.bacc as bacc
    import concourse.bass as bass
    import concourse.tile as tile
    from concourse import mybir

    f32 = mybir.dt.float32
    f16 = mybir.dt.float16
    Act = mybir.ActivationFunctionType

    nc = bacc.Bacc(
        "TRN2",
        target_bir_lowering=False,
        debug=False,
        enable_asserts=False,
        num_devices=NCORES,
    )

    xc_d = nc.dram_tensor("xc", [C + 1, XR, WE], f16, kind="ExternalInput").ap()
    wq_d = nc.dram_tensor("wq", [C + 1, C], f16, kind="ExternalInput").ap()
    wk_d = nc.dram_tensor("wk", [C + 1, C], f16, kind="ExternalInput").ap()
    wv_d = nc.dram_tensor("wv", [C + 1, C], f16, kind="ExternalInput").ap()
    rel_d = nc.dram_tensor("relv", [NT], f32, kind="ExternalInput").ap()
    out_d = nc.dram_tensor("outp", [2 * C, RPG, W], f32, kind="ExternalOutput").ap()

    from contextlib import ExitStack

    with tile.TileContext(nc) as tc:
        with ExitStack() as stk:
            wp = stk.enter_context(tc.tile_pool(name="wpool", bufs=1))
            mp = stk.enter_context(tc.tile_pool(name="main", bufs=1))
            tp = stk.enter_context(tc.tile_pool(name="tmp", bufs=2))
            pp = stk.enter_context(
                tc.tile_pool(name="psum", bufs=4, space=bass.MemorySpace.PSUM)
            )
            xp_cm = tc.tile_pool(name="xpool", bufs=1)
            xp = xp_cm.__enter__()

            # ---- loads ----
            xc = xp.tile([C + 1, XR, WE], f16)
            nc.sync.dma_start(out=xc[:, :KR, :], in_=xc_d[:, :KR, :])
            nc.sync.dma_start(out=xc[:, KR:, :], in_=xc_d[:, KR:, :])
            wq = wp.tile([C + 1, C], f16)
            wk = wp.tile([C + 1, C], f16)
            wv = wp.tile([C + 1, C], f16)
            nc.sync.dma_start(out=wq, in_=wq_d)
            nc.sync.dma_start(out=wk, in_=wk_d)
            nc.sync.dma_start(out=wv, in_=wv_d)
            relsb = wp.tile([128, NT], f32)
            nc.sync.dma_start(
                out=relsb, in_=rel_d.unsqueeze(0).broadcast_to([128, NT])
            )

            # ---- persistent tensors ----
            k2 = mp.tile([128, KR, WE], f16)    # partition = c + 64g
            v2 = mp.tile([128, KR, WE], f16)
            k2o = mp.tile([128, KR, WE], f16)   # shifted 1 col (fp16 alignment)
            v2o = mp.tile([128, KR, WE], f16)
            q2 = mp.tile([128, RPG, W], f16)
            qs = mp.tile([128, NS], f32)
            A = mp.tile([128, NT, NS], f32)     # logits+rel -> exp(wts)
            den = mp.tile([128, NS], f32)
            rden = mp.tile([128, NS], f32)
            wn16 = mp.tile([128, NT, NS], f16)  # normalized weights

            # ---- projections ----
            KVCH = 6
            kv_n = KR * WE // KVCH  # 510
            for dst, wgt in ((k2, wk), (v2, wv)):
                for ci in range(KVCH):
                    ps = pp.tile([128, 512], f32, tag="ps_kv")
                    for g in range(G):
                        rhs = (
                            xc[:, RPG * g : RPG * g + KR, :]
                            .rearrange("p a b -> p (a b)")[:, ci * kv_n : (ci + 1) * kv_n]
                        )
                        nc.tensor.matmul(
                            ps[64 * g : 64 * g + 64, :kv_n],
                            wgt,
                            rhs,
                            start=True,
                            stop=True,
                        )
                    dst_sl = dst.rearrange("p a b -> p (a b)")[
                        :, ci * kv_n : (ci + 1) * kv_n
                    ]
                    if ci % 2 == 0:
                        nc.vector.tensor_copy(dst_sl, ps[:, :kv_n])
                    else:
                        nc.scalar.copy(dst_sl, ps[:, :kv_n])
            QCH = 6
            qrows = RPG // QCH  # 4
            qn = qrows * W  # 384
            for ci in range(QCH):
                ps = pp.tile([128, 512], f32, tag="ps_q")
                for g in range(G):
                    r0 = HALO + RPG * g + ci * qrows
                    rhs = xc[:, r0 : r0 + qrows, HALO : HALO + W]
                    nc.tensor.matmul(
                        ps[64 * g : 64 * g + 64, :qn], wq, rhs, start=True, stop=True
                    )
                q2_sl = q2[:, ci * qrows : (ci + 1) * qrows, :]
                ps_sl = ps[:, :qn].rearrange("p (a b) -> p a b", b=W)
                if ci % 2 == 0:
                    nc.vector.tensor_copy(q2_sl, ps_sl)
                else:
                    nc.scalar.copy(q2_sl, ps_sl)

            # odd-shifted copies for fp16 alignment at odd kw
            nc.scalar.copy(
                k2o.rearrange("p a b -> p (a b)")[:, : KR * WE - 1],
                k2.rearrange("p a b -> p (a b)")[:, 1:],
            )
            nc.scalar.copy(
                v2o.rearrange("p a b -> p (a b)")[:, : KR * WE - 1],
                v2.rearrange("p a b -> p (a b)")[:, 1:],
            )

            # projections emitted; release x pool address space
            xp_cm.__exit__(None, None, None)
            abp = stk.enter_context(tc.tile_pool(name="abpool", bufs=1))

            # ---- qsum and rel outer product ----
            nc.vector.reduce_sum(
                out=qs,
                in_=q2.rearrange("p h (a b) -> p (h a) b", b=16),
                axis=mybir.AxisListType.X,
            )
            # ---- qk taps, processed in pairs: fp16 mults (2x) + shared 4-level
            # fp16 pairwise tree (2x), then per-tap fused logits STT ----
            pairs = []
            for kh in range(KS):
                for kw in range(0, KS - 1, 2):
                    pairs.append((kh * KS + kw, (kh, kw), (kh, kw + 1)))
                pairs.append((kh * KS + KS - 1, (kh, KS - 1), None))

            def ksl(src, kh, kw):
                s, o = (src[0], kw) if kw % 2 == 0 else (src[1], kw - 1)
                return s[:, kh : kh + RPG, o : o + W]

            for t0, tapa, tapb in pairs:
                nb = 2 if tapb else 1
                pr = tp.tile([128, 2, RPG, W], f16, tag="pr")
                nc.vector.tensor_mul(pr[:, 0], q2, ksl((k2, k2o), *tapa))
                if tapb:
                    nc.vector.tensor_mul(pr[:, 1], q2, ksl((k2, k2o), *tapb))
                prv = pr[:, :nb].rearrange("p t h (a b) -> p t (h a) b", b=16)
                t1 = tp.tile([128, 2, NS, 8], f16, tag="t1")
                nc.vector.tensor_add(t1[:, :nb], prv[:, :, :, 0:8], prv[:, :, :, 8:16])
                t2 = tp.tile([128, 2, NS, 4], f16, tag="t2")
                nc.vector.tensor_add(t2[:, :nb], t1[:, :nb, :, 0:4], t1[:, :nb, :, 4:8])
                t3 = tp.tile([128, 2, NS, 2], f16, tag="t3")
                nc.vector.tensor_add(t3[:, :nb], t2[:, :nb, :, 0:2], t2[:, :nb, :, 2:4])
                l4 = tp.tile([128, 2, NS], f32, tag="l4")
                nc.vector.tensor_add(l4[:, :nb], t3[:, :nb, :, 0], t3[:, :nb, :, 1])
                for i in range(nb):
                    nc.vector.scalar_tensor_tensor(
                        out=A[:, t0 + i, :],
                        in0=qs,
                        scalar=relsb[:, t0 + i : t0 + i + 1],
                        in1=l4[:, i],
                        op0=mybir.AluOpType.mult,
                        op1=mybir.AluOpType.add,
                    )
            # softmax in 2 site-halves so exp (ACT) overlaps den/wn (DVE);
            # no max-subtraction: |logits| < ~55, fp32-safe
            HS = NS // 2
            for h0 in (0, HS):
                Asl = A[:, :, h0 : h0 + HS]
                nc.scalar.activation(Asl, Asl, Act.Exp)
                nc.vector.reduce_sum(
                    out=den[:, h0 : h0 + HS],
                    in_=Asl.transpose([0, 2, 1]),
                    axis=mybir.AxisListType.X,
                )
                nc.vector.reciprocal(rden[:, h0 : h0 + HS], den[:, h0 : h0 + HS])
                nc.vector.tensor_mul(
                    wn16[:, :, h0 : h0 + HS],
                    Asl,
                    rden[:, h0 : h0 + HS].unsqueeze(1).broadcast_to([128, NT, HS]),
                )

            # ---- AV: expand wts (ACT), fp16 mult (2x), 3-level fp16 tree per
            # 8-tap group, groups chained (fp16, 2x) into an accumulator ----
            acc = mp.tile([128, RPG, W], f16)
            carry = {}  # tree level (0..2) -> partial-sum tile
            state = {"groups": 0}

            def tree_push(p, level=0):
                while level in carry and level < 3:
                    prev = carry.pop(level)
                    s = abp.tile(
                        [128, RPG, W], f16,
                        tag=f"ts{level}", name=f"ts{level}", bufs=2,
                    )
                    nc.vector.tensor_add(s, prev, p)
                    p = s
                    level += 1
                if level == 3:
                    if state["groups"] == 0:
                        nc.vector.tensor_copy(acc, p)
                    else:
                        nc.vector.tensor_add(acc, acc, p)
                    state["groups"] += 1
                else:
                    carry[level] = p

            for kh in range(KS):
                for kw in range(KS):
                    t = kh * KS + kw
                    vsrc, kwoff = (v2, kw) if kw % 2 == 0 else (v2o, kw - 1)
                    wexp = abp.tile(
                        [128, RPG, NB, 16], f16, tag="wexp", name="wexp", bufs=3
                    )
                    wsl = (
                        wn16[:, t, :]
                        .rearrange("p (h a) -> p h a", a=NB)
                        .unsqueeze(3)
                        .broadcast_to([128, RPG, NB, 16])
                    )
                    nc.scalar.copy(wexp, wsl)
                    p = abp.tile([128, RPG, W], f16, tag="avp", name="avp", bufs=3)
                    nc.vector.tensor_mul(
                        p,
                        wexp.rearrange("p h a b -> p h (a b)"),
                        vsrc[:, kh : kh + RPG, kwoff : kwoff + W],
                    )
                    tree_push(p)
            # leftover carries (49 = 6*8 + 1 -> a level-0 carry remains)
            for lv in sorted(carry):
                nc.vector.tensor_add(acc, acc, carry.pop(lv))
            # relu + cast fp16->fp32 and store, split in halves to drain early
            oute = mp.tile([128, RPG, W], f32)
            for r0 in (0, RPG // 2):
                nc.scalar.activation(
                    oute[:, r0 : r0 + RPG // 2, :],
                    acc[:, r0 : r0 + RPG // 2, :],
                    Act.Relu,
                )
                nc.sync.dma_start(
                    out=out_d[:, r0 : r0 + RPG // 2, :],
                    in_=oute[:, r0 : r0 + RPG // 2, :],
                )

    nc.compile()
    return nc


def _get_nc():
    if "nc" not in _cache:
        _cache["nc"] = _build()
    return _cache["nc"]


def _prep_inputs(inputs):
    """Host-side shard prep. Returns list of 8 in_maps."""
    x = np.ascontiguousarray(np.asarray(inputs["input_x"], dtype=np.float32))
    qw = np.asarray(inputs["q_w"], np.float32)
    qb = np.asarray(inputs["q_b"], np.float32)
    kw_ = np.asarray(inputs["k_w"], np.float32)
    kb = np.asarray(inputs["k_b"], np.float32)
    vw = np.asarray(inputs["v_w"], np.float32)
    vb = np.asarray(inputs["v_b"], np.float32)
    rh = np.asarray(inputs["rel_h"], np.float32).sum(0)[:, 0]  # (7,)
    rw = np.asarray(inputs["rel_w"], np.float32).sum(0)[0, :]  # (7,)

    wq = np.concatenate([qw.T, qb[None, :]], axis=0).astype(np.float16)  # (65, 64)
    wk = np.concatenate([kw_.T, kb[None, :]], axis=0).astype(np.float16)
    wv = np.concatenate([vw.T, vb[None, :]], axis=0).astype(np.float16)
    relv = (rh[:, None] + rw[None, :]).reshape(-1).astype(np.float32)  # (49,)

    # padded x with ones channel: (B, 65, 102, 102)
    xpad = np.zeros((B, C + 1, H + 2 * HALO, W + 2 * HALO), np.float16)
    xpad[:, :C, HALO : HALO + H, HALO : HALO + W] = x
    xpad[:, C, :, :] = 1.0

    in_maps = []
    for j in range(NCORES):
        b = j // 2
        r0 = RPC * (j % 2)
        xc = np.ascontiguousarray(xpad[b, :, r0 : r0 + XR, :])  # (65, 54, 102)
        in_maps.append({"xc": xc, "wq": wq, "wk": wk, "wv": wv, "relv": relv})
    return in_maps


def _assemble(results):
    """results: list of 8 dicts with 'outp' (128, 24, 96) -> (B, C, H, W)."""
    y = np.empty((B, C, H, W), np.float32)
    for j in range(NCORES):
        o = results[j]["outp"]
        b = j // 2
        r0 = RPC * (j % 2)
        for g in range(G):
            y[b, :, r0 + RPG * g : r0 + RPG * (g + 1), :] = o[64 * g : 64 * g + 64]
    return y


def _install_ntff_hook():
    """Register the axon NTFF profiling hook (the image lacks antenv.axon_hooks)."""
    import types
    import antenv

    if "antenv.axon_hooks" in sys.modules:
        return
    mod = types.ModuleType("antenv.axon_hooks")
    _state = {"hook": None}
    mod.set_axon_ntff_profile_hook = lambda h: _state.__setitem__("hook", h)
    mod.get_axon_ntff_profile_hook = lambda: _state["hook"]
    sys.modules["antenv.axon_hooks"] = mod
    antenv.axon_hooks = mod
    from trn_agent_boot.trn_boot import _ntff_profile_via_ctypes

    mod.set_axon_ntff_profile_hook(_ntff_profile_via_ctypes("/opt/axon/libaxon_pjrt.so"))
    # avoid S3 artifact uploads in-container
    from concourse import bass_utils

    bass_utils.upload_artifacts = lambda tmpdir: tmpdir


def kernel(**inputs) -> np.ndarray:
    from concourse import bass_utils

    nc = _get_nc()
    in_maps = _prep_inputs(inputs)
    trace = bool(int(os.environ.get("KERNEL_TRACE", "0")))
    kw = {}
    if trace:
        _install_ntff_hook()
        kw["tmpdir"] = os.environ.get("KERNEL_TRACE_DIR") or None
    res = bass_utils.run_bass_kernel_spmd(
        nc, in_maps, core_ids=list(range(NCORES)), trace=trace, **kw
    )
    _cache["last_result"] = res
    return _assemble(res.results)


def kernel_sim(inputs, cores=(0,)):
    """CoreSim-based check (no hardware). Returns partial output dict {core: outp}."""
    from concourse.bass_interp import CoreSim

    nc = _get_nc()
    in_maps = _prep_inputs(inputs)
    outs = {}
    for j in cores:
        sim = CoreSim(nc, trace=False, require_finite=True, require_nnan=True)
        for name, arr in in_maps[j].items():
            sim.tensor(name)[:] = arr
        sim.simulate(check_with_hw=False)
        outs[j] = np.array(sim.tensor("outp"))
    return outs
